# revision 1
# baseline (speedup 1.0000x reference)
"""EntityAttentionLayer on 8 Trainium2 NeuronCores (Bass/Tile).

Reference computation (per batch b of 1024):
    qkv = entities @ W_in.T            # [128 ents, 3*512]
    q (first 32 ents), k, v -> 8 heads x 64
    logits = q k^T / 8, masked by pre_mask (True = masked out)
    w = softmax(logits), fully-masked rows -> 0
    out = (w v) @ W_out.T + b_out, zeroed where post_mask

Sharding: data-parallel over batch, 128 batches per core.

Per-core kernel design:
  - The PE streams ~1 column/cycle at 2.4 GHz regardless of matmul shape
    (weight loads overlap via FWL when they fit under the previous stream),
    so the budget is total streamed columns: 49152/8-batch iteration here.
    QKV + out projections are minimal (full 128x128 tiles); the attention
    matmuls are shrunk as follows. (fp8 DoubleRow was measured to stream at
    the same ns/instruction as bf16 and uncompensated fp8 fails the accuracy
    budget, so everything stays bf16.)
  - logits: 2 heads per matmul via a block-diagonal q operand (lhsT
    [128=2x64 feats, 64=2x32 agents], zeros off-diagonal, zeroed once at
    startup in two persistent ping-pong tiles), streaming the shared k
    feature pair once: 4 matmuls x 128 cols per batch instead of 8 x 128.
    Two head-pair groups are column-packed per PSUM tile via tile_position
    (0, 64).
  - softmax: the pre_mask is applied additively (host sends -30*mask; W_q is
    pre-scaled by 1/8 so logits need no extra scale) with one in-place
    vector add per PSUM logits tile; Exp emits bf16 weights (scalar), one
    vector reduce + reciprocal_approx_fast + per-partition scale normalizes.
  - w is PE-transposed per (batch, head-pair-group) so attn@v contracts over
    entities; the out-projection runs as 4-ki accumulation groups with a
    fused (bias add) x (post-mask mul) scalar_tensor_tensor epilogue, and
    the bf16 output streams out per 256-feature half on the gpsimd queue.
  - engine balance: k copies split vector/scalar, v/attn/q copies + exp on
    scalar, softmax/epilogue on vector, input DMAs split across the sync and
    gpsimd queues; W_in is DMA'd per section (q, k, v) interleaved with
    iteration 0's input DMAs so compute starts as data arrives.
  - PSUM: 4-buffer ring for the projection matmuls, 2 logits banks, 1
    transpose bank, and one bank shared by the attn@v and out-projection
    accumulators (they are sequential users).
  - 16 batch-iterations are software-pipelined (iter N's projections overlap
    iter N-1's attention). The last iteration interleaves its own attention
    at 4-batch granularity (fine-chunked softmax on the drain tail) so the
    PE stays dense to the end, which also avoids the HAM duty-cycle throttle
    that otherwise halves PE rate once the pipeline goes sparse.
"""
import sys

sys.path.insert(0, "/opt/trn_rl_repo")

import numpy as np
import ml_dtypes

BS, NE, IN_DIM = 1024, 128, 512
EMBED, OUT_DIM = 512, 512
N_HEADS, N_AGENTS = 8, 32
HEAD_DIM = EMBED // N_HEADS  # 64
N_CORES = 8


def build_nc(b_core: int):
    """Build the per-core Bass program for b_core batches (b_core % 8 == 0)."""
    import concourse.bass as bass
    import concourse.tile as tile
    from concourse import bacc, mybir
    from concourse.masks import make_identity

    F32 = mybir.dt.float32
    BF16 = mybir.dt.bfloat16
    Exp = mybir.ActivationFunctionType.Exp
    Alu = mybir.AluOpType

    assert b_core % 8 == 0
    n_iter = b_core // 8

    nc = bacc.Bacc("TRN2", target_bir_lowering=False, debug=False)

    xt_d = nc.declare_dram_parameter("xt", [b_core, IN_DIM, NE], BF16, isOutput=False)
    xta_d = nc.declare_dram_parameter("xta", [4, 128, b_core, N_AGENTS], BF16, isOutput=False)
    wi_d = nc.declare_dram_parameter("wi", [IN_DIM, 3 * EMBED], BF16, isOutput=False)
    wo_d = nc.declare_dram_parameter("wo", [EMBED, OUT_DIM], BF16, isOutput=False)
    mneg_d = nc.declare_dram_parameter("mneg", [b_core, N_AGENTS, NE], BF16, isOutput=False)
    pkeep_d = nc.declare_dram_parameter("pkeep", [b_core, N_AGENTS], F32, isOutput=False)
    bias_d = nc.declare_dram_parameter("bias", [OUT_DIM], F32, isOutput=False)
    out_d = nc.declare_dram_parameter("out", [OUT_DIM, b_core, N_AGENTS], BF16, isOutput=True)

    AP = bass.AP

    def dram_ap(handle, offset, ap):
        base = handle[:]
        return AP(tensor=base.tensor, offset=offset, ap=ap)

    with tile.TileContext(nc) as tc:
        with (
            tc.tile_pool(name="const", bufs=1) as constp,
            tc.tile_pool(name="ins", bufs=2) as insp,
            tc.tile_pool(name="mid", bufs=2) as midp,
            tc.tile_pool(name="attn", bufs=2) as attnp,
            tc.tile_pool(name="outs", bufs=2) as outsp,
            tc.tile_pool(name="ps_mm", bufs=4, space="PSUM") as ps_mm,
            tc.tile_pool(name="ps_lg", bufs=1, space="PSUM") as ps_lg,
            tc.tile_pool(name="ps_wt", bufs=1, space="PSUM") as ps_wt,
            tc.tile_pool(name="ps_at", bufs=1, space="PSUM") as ps_at,
        ):
            # ---- constants ----
            wi_sb = constp.tile([128, 4, 3 * EMBED], BF16, name="wi_sb", tag="wi_sb")
            wo_sb = constp.tile([128, 4, OUT_DIM], BF16)
            bias_sb = constp.tile([128, 4], F32)
            ident = constp.tile([128, 128], BF16)
            # block-diagonal q operands, double-buffered manually; off-diagonal
            # zero blocks are written once and never touched again
            qbd = [
                constp.tile([128, 4, 8, 64], BF16, name=f"qbd_{i}", tag=f"qbd_{i}")
                for i in range(2)
            ]

            def emit_wi_section(sec, mo=None):
                # DMA one section (0=q, 1=k, 2=v) of W_in (all 4 ki); per-mo
                # 128-col slices let iter-0 compute start as data arrives
                c0 = sec * EMBED + (0 if mo is None else mo * 128)
                nc2 = EMBED if mo is None else 128
                nc.sync.dma_start(
                    out=wi_sb[:, :, c0 : c0 + nc2],
                    in_=dram_ap(
                        wi_d, c0,
                        [[3 * EMBED, 128], [128 * 3 * EMBED, 4], [1, nc2]],
                    ),
                )

            def emit_late_consts():
                nc.sync.dma_start(
                    out=wo_sb,
                    in_=dram_ap(wo_d, 0, [[OUT_DIM, 128], [128 * OUT_DIM, 4], [1, OUT_DIM]]),
                )
                nc.sync.dma_start(out=bias_sb, in_=dram_ap(bias_d, 0, [[1, 128], [128, 4]]))
                make_identity(nc, ident)
                for i in range(2):
                    nc.gpsimd.memset(qbd[i][0:64, :, :, 32:64], 0.0)
                    nc.gpsimd.memset(qbd[i][64:128, :, :, 0:32], 0.0)

            def emit_inputs(it, interleave=None):
                """Issue this iter's input DMAs; returns the state dict.
                interleave: optional callback list run between DMA groups
                (used for iter 0 to start compute ASAP)."""
                b0 = it * 8
                st = {"it": it, "qbd": qbd[it % 2]}
                st["xta"] = xta_sb = insp.tile(
                    [128, 4, 8, N_AGENTS], BF16, name="xta_sb", tag="xta_sb"
                )
                nc.gpsimd.dma_start(
                    out=xta_sb,
                    in_=dram_ap(
                        xta_d,
                        b0 * N_AGENTS,
                        [[b_core * N_AGENTS, 128], [128 * b_core * N_AGENTS, 4],
                         [N_AGENTS, 8], [1, N_AGENTS]],
                    ),
                )
                if interleave:
                    interleave.pop(0)()
                st["xt"] = xt_sb = insp.tile([128, 4, 8, NE], BF16, name="xt_sb", tag="xt_sb")
                for g2 in range(2):
                    for ki in range(4):
                        nc.sync.dma_start(
                            out=xt_sb[:, ki, g2 * 4 : (g2 + 1) * 4, :],
                            in_=dram_ap(
                                xt_d,
                                (b0 + g2 * 4) * IN_DIM * NE + ki * 128 * NE,
                                [[NE, 128], [IN_DIM * NE, 4], [1, NE]],
                            ),
                        )
                    if interleave:
                        interleave.pop(0)()
                # additive pre-mask (-30 where masked), replicated over the 4
                # head-pair partition groups
                st["mneg"] = mneg_bc = insp.tile(
                    [128, 8, NE], BF16, name="mneg_bc", tag="mneg_bc"
                )
                for cg in range(4):
                    nc.gpsimd.dma_start(
                        out=mneg_bc[cg * 32 : (cg + 1) * 32, :, :],
                        in_=dram_ap(
                            mneg_d,
                            b0 * N_AGENTS * NE,
                            [[NE, 32], [N_AGENTS * NE, 8], [1, NE]],
                        ),
                    )
                st["pkeep"] = pkeep_bc = insp.tile(
                    [128, 8, N_AGENTS], F32, name="pkeep_bc", tag="pkeep_bc"
                )
                nc.gpsimd.dma_start(
                    out=pkeep_bc,
                    in_=dram_ap(pkeep_d, b0 * N_AGENTS, [[0, 128], [N_AGENTS, 8], [1, N_AGENTS]]),
                )
                st["kt"] = midp.tile([128, 4, 8, NE], BF16, name="kt_sb", tag="kt_sb")
                st["vt"] = midp.tile([128, 8, EMBED], BF16, name="vt_sb", tag="vt_sb")
                return st

            def emit_q_unit(st, mo):
                q_ps = ps_mm.tile([128, 8, N_AGENTS], F32, tag="mm", name="q_ps")
                for ki in range(4):
                    nc.tensor.matmul(
                        q_ps,
                        wi_sb[:, ki, mo * 128 : (mo + 1) * 128],
                        st["xta"][:, ki, :, :],
                        start=(ki == 0),
                        stop=(ki == 3),
                    )
                # scatter into the block-diagonal operand (2 heads per mo)
                qb = st["qbd"]
                nc.scalar.copy(out=qb[0:64, mo, :, 0:32], in_=q_ps[0:64, :, :])
                nc.vector.tensor_copy(out=qb[64:128, mo, :, 32:64], in_=q_ps[64:128, :, :])

            def emit_k_unit(st, mo, g2):
                k_ps = ps_mm.tile([128, 4, NE], F32, tag="mm", name="k_ps")
                for ki in range(4):
                    nc.tensor.matmul(
                        k_ps,
                        wi_sb[:, ki, EMBED + mo * 128 : EMBED + (mo + 1) * 128],
                        st["xt"][:, ki, g2 * 4 : (g2 + 1) * 4, :],
                        start=(ki == 0),
                        stop=(ki == 3),
                    )
                if g2 == 0:
                    nc.vector.tensor_copy(
                        out=st["kt"][:, mo, g2 * 4 : (g2 + 1) * 4, :], in_=k_ps
                    )
                else:
                    nc.scalar.copy(out=st["kt"][:, mo, g2 * 4 : (g2 + 1) * 4, :], in_=k_ps)

            def emit_v_unit(st, b):
                v_ps = ps_mm.tile([128, EMBED], F32, tag="mm", name="v_ps")
                for ki in range(4):
                    nc.tensor.matmul(
                        v_ps,
                        st["xt"][:, ki, b, :],
                        wi_sb[:, ki, 2 * EMBED : 3 * EMBED],
                        start=(ki == 0),
                        stop=(ki == 3),
                    )
                nc.scalar.copy(out=st["vt"][:, b, :], in_=v_ps)

            def emit_softmax(st, sc, fine=False):
                """logits (block-diag, 2 heads/matmul) + additive mask + exp
                + row sums + normalize, for 4 batches. fine=True chunks the
                vector/scalar work per (bs, g) for lower chain latency (used
                on the drain tail)."""
                qb, kt = st["qbd"], st["kt"]
                lg = [
                    ps_lg.tile([128, 4, NE], F32, tag="lg0", name="lg0"),
                    ps_lg.tile([128, 4, NE], F32, tag="lg1", name="lg1"),
                ]  # [(hp2, h2, a), bs, e] for head-pair-group g = 0, 1
                for bs in range(4):
                    b = sc * 4 + bs
                    for g in range(2):
                        for hp2 in range(2):
                            hp = g * 2 + hp2
                            nc.tensor.matmul(
                                lg[g][hp2 * 64 : (hp2 + 1) * 64, bs, :],
                                qb[:, hp, b, :],
                                kt[:, hp, b, :],
                                start=True,
                                stop=True,
                                tile_position=(0, hp2 * 64),
                            )
                we = attnp.tile([128, 4, 2, NE], BF16, name="we", tag="we")
                sums = attnp.tile([128, 4, 2], F32, name="sums", tag="sums")
                rcp = attnp.tile([128, 4, 2], F32, name="rcp", tag="rcp")
                wn = attnp.tile([128, 4, 2, NE], BF16, name="wn", tag="wn")
                if fine:
                    for bs in range(4):
                        for g in range(2):
                            nc.vector.tensor_add(
                                lg[g][:, bs, :], lg[g][:, bs, :],
                                st["mneg"][:, sc * 4 + bs, :],
                            )
                            nc.scalar.activation(
                                out=we[:, bs, g, :], in_=lg[g][:, bs, :], func=Exp
                            )
                            nc.vector.reduce_sum(
                                sums[:, bs, g : g + 1], we[:, bs, g, :],
                                axis=mybir.AxisListType.X,
                            )
                            nc.vector.reciprocal_approx_fast(
                                out=rcp[:, bs, g : g + 1], in_=sums[:, bs, g : g + 1]
                            )
                            nc.vector.tensor_scalar_mul(
                                wn[:, bs, g, :], we[:, bs, g, :], rcp[:, bs, g : g + 1]
                            )
                else:
                    for g in range(2):
                        nc.vector.tensor_add(
                            lg[g], lg[g], st["mneg"][:, sc * 4 : (sc + 1) * 4, :]
                        )
                    for g in range(2):
                        nc.scalar.activation(out=we[:, :, g, :], in_=lg[g], func=Exp)
                    nc.vector.reduce_sum(sums, we, axis=mybir.AxisListType.X)
                    nc.vector.reciprocal_approx_fast(out=rcp, in_=sums)
                    for bs in range(4):
                        for g in range(2):
                            nc.vector.tensor_scalar_mul(
                                wn[:, bs, g, :],
                                we[:, bs, g, :],
                                rcp[:, bs, g : g + 1],
                            )
                st[f"wn{sc}"] = wn

            def emit_attnv(st, sc, attn_sb, fillers=None):
                """transpose w + attn@v for 4 batches; writes attn_sb bf16.
                The attn@v matmuls are weight-load-bound (64-col loads over
                32-col streams), so `fillers` (stream-bound matmul closures,
                e.g. the out-projection of the other subchunk) are popped one
                per head-pair to hide the loads under their streams."""
                wn, vt = st[f"wn{sc}"], st["vt"]
                wt_ps = ps_wt.tile([128, 4, 2, NE], BF16, name="wt_ps")  # [e, bs, g, (hp2,h2,a)]
                for bs in range(4):
                    for g in range(2):
                        nc.tensor.transpose(wt_ps[:, bs, g, :], wn[:, bs, g, :], ident)
                wt_sb = attnp.tile([128, 4, 2, NE], BF16, name="wt_sb", tag="wt_sb")
                nc.vector.tensor_copy(out=wt_sb, in_=wt_ps)
                at_ps = ps_at.tile([128, 4, 4, N_AGENTS], F32, name="at_ps", tag="atop")
                for bs in range(4):
                    b = sc * 4 + bs
                    for hp in range(4):
                        g, hp2 = hp // 2, hp % 2
                        for h2 in range(2):
                            h = hp * 2 + h2
                            nc.tensor.matmul(
                                at_ps[h2 * 64 : h2 * 64 + 64, bs, hp, :],
                                vt[:, b, h * 64 : (h + 1) * 64],
                                wt_sb[:, bs, g, hp2 * 64 + h2 * 32 : hp2 * 64 + (h2 + 1) * 32],
                                start=True,
                                stop=True,
                                tile_position=(0, h2 * 64),
                            )
                        if fillers:
                            fillers.pop(0)()
                while fillers:
                    fillers.pop(0)()
                nc.scalar.copy(out=attn_sb[:, sc * 4 : (sc + 1) * 4, :, :], in_=at_ps)

            def emit_outdma(st, out_sb, mh):
                # stream a 256-feature half out
                nc.gpsimd.dma_start(
                    out=dram_ap(
                        out_d,
                        mh * 2 * 128 * b_core * N_AGENTS + st["it"] * 8 * N_AGENTS,
                        [[b_core * N_AGENTS, 128],
                         [128 * b_core * N_AGENTS, 2],
                         [N_AGENTS, 8],
                         [1, N_AGENTS]],
                    ),
                    in_=out_sb[:, mh * 2 : mh * 2 + 2, :, :],
                )

            def outproj_ops(st, attn_sb, out_sb, sc):
                """Closures for one subchunk's out-projection: 16 matmuls +
                2 epilogue ops, poppable one at a time for interleaving."""
                bs0, nbs = (0, 8) if sc is None else (sc * 4, 4)
                cell = {}
                ops = []

                def mm(mh, m2, ki2):
                    if m2 == 0 and ki2 == 0:
                        cell[mh] = ps_at.tile(
                            [128, 2, nbs, N_AGENTS], F32, name="op_ps", tag="atop"
                        )
                    mo2 = mh * 2 + m2
                    nc.tensor.matmul(
                        cell[mh][:, m2, :, :],
                        wo_sb[:, ki2, mo2 * 128 : (mo2 + 1) * 128],
                        attn_sb[:, bs0 : bs0 + nbs, ki2, :],
                        start=(ki2 == 0),
                        stop=(ki2 == 3),
                    )

                def stt(mh):
                    for m2 in range(2):
                        mo2 = mh * 2 + m2
                        nc.vector.scalar_tensor_tensor(
                            out=out_sb[:, mo2, bs0 : bs0 + nbs, :],
                            in0=cell[mh][:, m2, :, :],
                            scalar=bias_sb[:, mo2 : mo2 + 1],
                            in1=st["pkeep"][:, bs0 : bs0 + nbs, :],
                            op0=Alu.add,
                            op1=Alu.mult,
                        )

                for mh in range(2):
                    for m2 in range(2):
                        for ki2 in range(4):
                            ops.append(lambda mh=mh, m2=m2, ki2=ki2: mm(mh, m2, ki2))
                    ops.append(lambda mh=mh: stt(mh))
                return ops

            def emit_outproj(st, attn_sb, out_sb, sc, dma=False):
                for i, op in enumerate(outproj_ops(st, attn_sb, out_sb, sc)):
                    op()
                    if dma and i == 8:  # mh=0 epilogue done
                        emit_outdma(st, out_sb, 0)
                if dma:
                    emit_outdma(st, out_sb, 1)

            def qkv_units(st, head=False):
                qs = [lambda mo=mo: emit_q_unit(st, mo) for mo in range(4)]
                ks = lambda g2: [
                    lambda mo=mo: emit_k_unit(st, mo, g2) for mo in range(4)
                ]
                vs = lambda lo: [lambda b=b: emit_v_unit(st, b) for b in range(lo, lo + 4)]
                if head:
                    # iter 0: consume strictly in DMA-arrival order
                    return qs + ks(0) + vs(0) + ks(1) + vs(4)
                units = qs
                for mo in range(4):
                    units.append(lambda mo=mo: emit_k_unit(st, mo, 0))
                    units.append(lambda mo=mo: emit_k_unit(st, mo, 1))
                    units.append(lambda b=2 * mo: emit_v_unit(st, b))
                    units.append(lambda b=2 * mo + 1: emit_v_unit(st, b))
                return units + vs(4)

            def attn_units(st):
                attn_sb = outsp.tile(
                    [128, 8, 4, N_AGENTS], BF16, name="attn_sb", tag="attn_sb"
                )
                out_sb = outsp.tile(
                    [128, 4, 8, N_AGENTS], BF16, name="out_sb", tag="out_sb"
                )
                st["attn_sb"] = attn_sb
                sm = [lambda sc=sc: emit_softmax(st, sc) for sc in range(2)]
                av = [lambda sc=sc: emit_attnv(st, sc, attn_sb) for sc in range(2)]
                op = lambda: emit_outproj(st, attn_sb, out_sb, None, dma=True)
                return [sm[0], av[0], sm[1], av[1], op]

            # software pipeline: interleave iter N's QKV with iter N-1's attention
            emit_wi_section(0)
            st0 = emit_inputs(
                0,
                interleave=[lambda: emit_wi_section(1), lambda: None,
                            lambda: emit_wi_section(2)],
            )
            emit_late_consts()
            prev = None
            for it in range(n_iter - 1):
                st = st0 if it == 0 else emit_inputs(it)
                qu = qkv_units(st)
                au = attn_units(prev) if prev is not None else []
                for i, u in enumerate(qu):
                    u()
                    if i % 4 == 3 and au:
                        au.pop(0)()
                for u in au:
                    u()
                prev = st
            # last iteration: interleave its own attention at 4-batch (sc)
            # granularity, keeping PE work (V b4-7, sc0 out-projection) in
            # flight under the final softmax chain so the drain tail is short
            st = emit_inputs(n_iter - 1) if n_iter > 1 else st0
            au_prev = attn_units(prev) if prev is not None else []
            attn_sb = outsp.tile(
                [128, 8, 4, N_AGENTS], BF16, name="attn_sb", tag="attn_sb"
            )
            out_sb = outsp.tile(
                [128, 4, 8, N_AGENTS], BF16, name="out_sb", tag="out_sb"
            )
            st["attn_sb"] = attn_sb
            units = (
                [lambda mo=mo: emit_q_unit(st, mo) for mo in range(4)]
                + [lambda mo=mo: emit_k_unit(st, mo, 0) for mo in range(4)]
                + [lambda b=b: emit_v_unit(st, b) for b in range(4)]
            )
            for i, u in enumerate(units):
                u()
                if i % 2 == 1 and au_prev:
                    au_prev.pop(0)()
            for u in au_prev:
                u()
            emit_softmax(st, 0)
            for mo in range(4):
                emit_k_unit(st, mo, 1)
            emit_softmax(st, 1, fine=True)
            emit_attnv(st, 0, attn_sb)
            for b in range(4, 8):
                emit_v_unit(st, b)
            emit_outproj(st, attn_sb, out_sb, 0, dma=False)
            emit_attnv(st, 1, attn_sb)
            emit_outproj(st, attn_sb, out_sb, 1, dma=True)

    nc.compile()
    return nc


def _prep_core_inputs(ents, mneg, pkeep, wi, wo, bias):
    """Host-side layout prep for one core's batch shard."""
    b_core = ents.shape[0]
    xt = np.ascontiguousarray(ents.transpose(0, 2, 1))  # [b, in, e]
    xta = np.ascontiguousarray(
        ents[:, :N_AGENTS, :].transpose(2, 0, 1)
    ).reshape(4, 128, b_core, N_AGENTS)
    return {
        "xt": xt,
        "xta": xta,
        "wi": wi,
        "wo": wo,
        "mneg": mneg,
        "pkeep": pkeep,
        "bias": bias,
    }


def run(entities, pre_mask, post_mask, W_in, W_out, b_out, trace=False):
    """Shard, run on 8 cores, gather. Returns (out, BassKernelResults)."""
    from concourse.bass_utils import run_bass_kernel_spmd

    bs = entities.shape[0]
    b_core = bs // N_CORES
    entities = np.asarray(entities, dtype=np.float32).astype(ml_dtypes.bfloat16)
    mneg = (np.asarray(pre_mask) * np.float32(-30.0)).astype(ml_dtypes.bfloat16)
    pkeep = (~np.asarray(post_mask)).astype(np.float32)
    wi_f = np.ascontiguousarray(np.asarray(W_in, dtype=np.float32).T)
    wi_f[:, :EMBED] *= np.float32(0.125)  # fold the 1/sqrt(head_dim) into W_q
    wi = wi_f.astype(ml_dtypes.bfloat16)
    wo = np.ascontiguousarray(np.asarray(W_out, dtype=np.float32).T).astype(ml_dtypes.bfloat16)
    bias = np.asarray(b_out, dtype=np.float32)

    nc = build_nc(b_core)
    in_maps = [
        _prep_core_inputs(
            entities[c * b_core : (c + 1) * b_core],
            mneg[c * b_core : (c + 1) * b_core],
            pkeep[c * b_core : (c + 1) * b_core],
            wi, wo, bias,
        )
        for c in range(N_CORES)
    ]
    res = run_bass_kernel_spmd(nc, in_maps, list(range(N_CORES)), trace=trace)
    out = np.empty((bs, N_AGENTS, OUT_DIM), dtype=np.float32)
    for c in range(N_CORES):
        out[c * b_core : (c + 1) * b_core] = (
            res.results[c]["out"].astype(np.float32).transpose(1, 2, 0)
        )
    return out, res


def kernel(entities, pre_mask, post_mask, W_in, W_out, b_out):
    out, _ = run(entities, pre_mask, post_mask, W_in, W_out, b_out, trace=False)
    return out

